# revision 24
# baseline (speedup 1.0000x reference)
"""GAT (Cora-style) forward pass on 8 Trainium2 NeuronCores via Bass/Tile.

Sharding: row-parallel (sequence parallel over target nodes). Each core owns
N/8 = 512 target rows for ALL 8 heads. Tunnel traffic is minimized:
  - x is shipped transposed fp16, sharded by node rows (0.5 MB/core)
  - adj is shipped bit-packed along the source axis (64 KB... 256 KB/core)
  - W is shipped fp16 sharded by head (64 KB/core) and AllGather'd on device
  - per-core Wh / t are AllGather'd on device (no replicated host shipping)
Output comes back fp16 [512, 512] per core (0.5 MB/core).

Device pipeline per core (scores in [target r = partition, source j = free]):
  Wh_loc[r,(h,d)] = x_c @ W[h]            (PE, fp16 -> fp32 psum)
  AllGather Wh -> Wh_all[j,(h,d)] fp16 ; s/t = Wh . a_src/a_dst (DVE)
  AllGather t  -> t_all[j,h] fp32 ; T_b[h] = partition-broadcast of t (DMA)
  per (h, rt):      E1 = exp(T_b + s_r)   (ACT, bias=per-partition s column)
                    E2 = exp(0.2*T_b + 0.2*s_r)
                    P  = max(E1, E2)       == exp(leakyrelu(s+t, 0.2))
                    Pm = P * mask          (DVE, unpacked adj bits; bit value
                                            2^(7-k) is a per-j scale that
                                            cancels between num and denom)
                    per jt: pmT = PE-transpose(Pm chunk)
                            out[r,d] += pmT^T @ [Wh_all | 1]  (ones -> denom)
  epilogue: out = elu(num * recip(denom)) -> fp16
"""

import numpy as np
from contextlib import ExitStack

N = 4096
F_IN = 512
H = 8
D = 64
NCORES = 8
R = N // NCORES          # 512 target rows per core
P = 128                  # partitions
RT = R // P              # 4 row tiles per core
JT = N // P              # 32 source-node tiles
FT = F_IN // P           # 4 feature tiles
QJ = N // 8              # 512 packed bytes per target row

_cached = None
_exec_cached = None


def _build_program():
    import concourse.bass as bass
    import concourse.tile as tile
    from concourse import mybir
    from concourse.masks import make_identity

    f16, f32, u8 = mybir.dt.float16, mybir.dt.float32, mybir.dt.uint8
    i8 = mybir.dt.int8
    AF = mybir.ActivationFunctionType
    OP = mybir.AluOpType

    nc = bass.Bass(trn_type="TRN2", num_devices=NCORES)

    xt_in = nc.dram_tensor("xt", [F_IN, R], f16, kind="ExternalInput")
    mb_in = nc.dram_tensor("mb", [R, QJ], u8, kind="ExternalInput")
    wc_in = nc.dram_tensor("wc", [F_IN, D], f16, kind="ExternalInput")
    asrc_in = nc.dram_tensor("asrc", [1, H * D], f16, kind="ExternalInput")
    adst_in = nc.dram_tensor("adst", [1, H * D], f16, kind="ExternalInput")
    out_ext = nc.dram_tensor("out", [R, H * D], i8, kind="ExternalOutput")
    osc_ext = nc.dram_tensor("oscale", [R, H], f16, kind="ExternalOutput")

    w_loc = nc.dram_tensor("w_loc", [F_IN, D], f16)
    w_all = nc.dram_tensor("w_all", [N, D], f16, addr_space="Shared")
    wh_loc = nc.dram_tensor("wh_loc", [R, H * D], f16)
    wh_all = nc.dram_tensor("wh_all", [N, H * D], f16, addr_space="Shared")
    t_loc = nc.dram_tensor("t_loc", [H, R], f32)
    t_all = nc.dram_tensor("t_all", [NCORES * H, R], f32, addr_space="Shared")

    groups = [list(range(NCORES))]

    with tile.TileContext(nc) as tc:
        with ExitStack() as ctx:
            statics = ctx.enter_context(tc.tile_pool(name="statics", bufs=1))
            work = ctx.enter_context(tc.tile_pool(name="work", bufs=3))
            small = ctx.enter_context(tc.tile_pool(name="small", bufs=4))

            # ---- Phase A: W gather, Wh, s/t, mask unpack ----
            nc.gpsimd.dma_start(w_loc[:], wc_in[:])
            nc.gpsimd.collective_compute(
                "AllGather", OP.bypass, replica_groups=groups,
                ins=[w_loc[:]], outs=[w_all[:]],
            )

            xt_s = statics.tile([P, FT, R], f16)
            nc.sync.dma_start(xt_s[:], xt_in[:].rearrange("(ft p) r -> p ft r", p=P))
            w_s = statics.tile([P, H * FT, D], f16)
            nc.sync.dma_start(w_s[:], w_all[:].rearrange("(a p) d -> p a d", p=P))

            wh_s = statics.tile([P, RT, H * D], f16)

            with tc.tile_pool(name="psumA", bufs=2, space="PSUM") as psumA:
                for rt in range(RT):
                    for h in range(H):
                        pwh = psumA.tile([P, D], f32, tag="pwh")
                        for ft in range(FT):
                            nc.tensor.matmul(
                                pwh[:],
                                lhsT=xt_s[:, ft, rt * P:(rt + 1) * P],
                                rhs=w_s[:, h * FT + ft, :],
                                start=(ft == 0), stop=(ft == FT - 1),
                            )
                        nc.scalar.activation(
                            wh_s[:, rt, h * D:(h + 1) * D], pwh[:], AF.Copy)

                nc.sync.dma_start(
                    wh_loc[:].rearrange("(rt p) c -> p rt c", p=P), wh_s[:])
                nc.gpsimd.collective_compute(
                    "AllGather", OP.bypass, replica_groups=groups,
                    ins=[wh_loc[:]], outs=[wh_all[:]],
                )

                # broadcast a_src/a_dst across partitions via stride-0 DMA
                asb = statics.tile([P, H * D], f16)
                nc.gpsimd.dma_start(asb[:], asrc_in[:].broadcast_to([P, H * D]))
                adb = statics.tile([P, H * D], f16)
                nc.gpsimd.dma_start(adb[:], adst_in[:].broadcast_to([P, H * D]))

                # local s/t: multiply Wh by broadcast a, segment-reduce over d
                s_loc = statics.tile([P, RT, H], f32)
                t_loc_s = statics.tile([P, RT, H], f32)
                for rt in range(RT):
                    for ab, dst in ((asb, s_loc), (adb, t_loc_s)):
                        tmp = work.tile([P, H * D], f32, tag="sttmp", bufs=2)
                        nc.vector.tensor_mul(tmp[:], wh_s[:, rt, :], ab[:])
                        for h in range(H):
                            nc.vector.tensor_reduce(
                                dst[:, rt, h:h + 1], tmp[:, h * D:(h + 1) * D],
                                axis=mybir.AxisListType.X, op=OP.add)

                s2_loc = statics.tile([P, RT, H], f32)
                nc.vector.tensor_scalar(
                    out=s2_loc[:], in0=s_loc[:], scalar1=0.2, scalar2=None,
                    op0=OP.mult)

                for rt in range(RT):
                    nc.gpsimd.dma_start(
                        t_loc[:, rt * P:(rt + 1) * P].rearrange("h p -> p h"),
                        t_loc_s[:, rt, :])
                nc.gpsimd.collective_compute(
                    "AllGather", OP.bypass, replica_groups=groups,
                    ins=[t_loc[:]], outs=[t_all[:]],
                )

            # mask unpack: bit k of byte q -> source j = 8q + k, normalized
            # to exactly {0,1} via (byte & (1<<(7-k))) >> (7-k) — both ops
            # bitwise-class, which walrus accepts as a dual-op tensor_scalar.
            mb_s = statics.tile([P, RT, QJ], u8)
            nc.sync.dma_start(
                mb_s[:], mb_in[:].rearrange("(rt p) q -> p rt q", p=P))
            mask_s = statics.tile([P, RT, QJ, 8], f16)
            for rt in range(RT):
                mu = work.tile([P, QJ, 8], u8, tag="mu", bufs=2)
                for k in range(8):
                    nc.vector.tensor_scalar(
                        out=mu[:, :, k], in0=mb_s[:, rt, :],
                        scalar1=1 << (7 - k), scalar2=7 - k,
                        op0=OP.bitwise_and, op1=OP.logical_shift_right)
                nc.vector.tensor_copy(
                    mask_s[:, rt].rearrange("p a b -> p (a b)"),
                    mu[:].rearrange("p q k -> p (q k)"))

            # Wh_all + ones column -> whp
            whp = statics.tile([P, JT, H, D + 1], f16)
            wh_all_v = wh_all[:].rearrange("(jt p) (h d) -> p jt h d", p=P, d=D)
            for jt in range(JT):
                nc.sync.dma_start(whp[:, jt, :, 0:D], wh_all_v[:, jt])
            nc.vector.memset(whp[:, :, :, D:D + 1], 1.0)

            ident16 = statics.tile([P, P], f16)
            make_identity(nc, ident16)

            out_sb = [statics.tile([P, H * D], i8, name=f"outsb{rt}",
                                   tag=f"outsb{rt}") for rt in range(RT)]
            osc_sb = [statics.tile([P, H], f16, name=f"oscsb{rt}",
                                   tag=f"oscsb{rt}") for rt in range(RT)]

            # ---- Phase B: scores + aggregation ----
            with tc.tile_pool(name="psumB", bufs=2, space="PSUM") as psumB:
                for h in range(H):
                    tb = work.tile([P, N], f32, tag="tb", bufs=2)
                    t_all_v = t_all[:].rearrange("(c h) r -> h c r", h=H)
                    nc.gpsimd.dma_start(
                        tb[:].rearrange("p (c r) -> p c r", c=NCORES),
                        t_all_v[h:h + 1].broadcast_to([P, NCORES, R]))
                    for rt in range(RT):
                        e1 = work.tile([P, N], f16, tag="e1", bufs=2)
                        nc.scalar.activation(
                            e1[:], tb[:], AF.Exp,
                            bias=s_loc[:, rt, h:h + 1], scale=1.0)
                        e2 = work.tile([P, N], f16, tag="e2", bufs=2)
                        nc.scalar.activation(
                            e2[:], tb[:], AF.Exp,
                            bias=s2_loc[:, rt, h:h + 1], scale=0.2)
                        pm = work.tile([P, N], f16, tag="pm", bufs=2)
                        nc.vector.tensor_tensor(
                            out=pm[:], in0=e1[:], in1=e2[:], op=OP.max)
                        nc.vector.tensor_mul(
                            pm[:], pm[:],
                            mask_s[:, rt].rearrange("p a b -> p (a b)"))
                        po = psumB.tile([P, D + 1], f32, tag="po")
                        for jt in range(JT):
                            ptp = psumB.tile([P, P], f16, tag="ptp")
                            nc.tensor.transpose(
                                ptp[:], pm[:, jt * P:(jt + 1) * P], ident16[:])
                            pmT = work.tile([P, P], f16, tag="pmT")
                            nc.scalar.activation(pmT[:], ptp[:], AF.Copy)
                            nc.tensor.matmul(
                                po[:], lhsT=pmT[:], rhs=whp[:, jt, h, :],
                                start=(jt == 0), stop=(jt == JT - 1),
                            )
                        # epilogue: out = elu(num/denom)
                        ocp = small.tile([P, D + 1], f32, tag="ocp")
                        nc.scalar.activation(ocp[:], po[:], AF.Copy)
                        rec = small.tile([P, 1], f32, tag="rec")
                        nc.vector.reciprocal(rec[:], ocp[:, D:D + 1])
                        z = small.tile([P, D], f32, tag="z")
                        nc.vector.tensor_scalar(
                            out=z[:], in0=ocp[:, 0:D], scalar1=rec[:],
                            scalar2=None, op0=OP.mult)
                        zn = small.tile([P, D], f32, tag="zn")
                        nc.vector.tensor_scalar(
                            out=zn[:], in0=z[:], scalar1=0.0, scalar2=None,
                            op0=OP.min)
                        en = small.tile([P, D], f32, tag="en")
                        nc.scalar.activation(en[:], zn[:], AF.Exp)
                        zp = small.tile([P, D], f32, tag="zp")
                        nc.vector.tensor_scalar(
                            out=zp[:], in0=z[:], scalar1=0.0, scalar2=None,
                            op0=OP.max)
                        # of = (en - 1) + zp  == elu(z), fp32
                        of = small.tile([P, D], f32, tag="of")
                        nc.vector.scalar_tensor_tensor(
                            out=of[:], in0=en[:], scalar=-1.0, in1=zp[:],
                            op0=OP.add, op1=OP.add)
                        # per-(row,head) int8 quantization: q = of*126/amax,
                        # shipped with scale amax/126 (126 avoids saturation)
                        amax = small.tile([P, 1], f32, tag="amax")
                        nc.vector.tensor_reduce(
                            amax[:], of[:], axis=mybir.AxisListType.X,
                            op=OP.max, apply_absolute_value=True)
                        am2 = small.tile([P, 1], f32, tag="am2")
                        nc.vector.tensor_scalar(
                            out=am2[:], in0=amax[:], scalar1=1.0 / 126.0,
                            scalar2=1e-20, op0=OP.mult, op1=OP.max)
                        sinv = small.tile([P, 1], f32, tag="sinv")
                        nc.vector.reciprocal(sinv[:], am2[:])
                        nc.vector.tensor_scalar(
                            out=out_sb[rt][:, h * D:(h + 1) * D], in0=of[:],
                            scalar1=sinv[:], scalar2=None, op0=OP.mult)
                        nc.scalar.activation(
                            osc_sb[rt][:, h:h + 1], am2[:], AF.Copy)

            out_view = out_ext[:].rearrange("(rt p) c -> p rt c", p=P)
            osc_view = osc_ext[:].rearrange("(rt p) c -> p rt c", p=P)
            for rt in range(RT):
                nc.sync.dma_start(out_view[:, rt, :], out_sb[rt][:])
                nc.gpsimd.dma_start(osc_view[:, rt, :], osc_sb[rt][:])

    _legalize_waits(nc, mybir)
    return nc


def _legalize_waits(nc, mybir, max_waits=1):
    """walrus codegen allows very few sync-wait commands per instruction.
    Split surplus waits onto same-engine NoOps inserted just before the
    instruction — same wait point in that engine's program order."""
    uid = 0
    for bb in nc.main_func.blocks:
        out = []
        for ins in bb.instructions:
            si = ins.sync_info
            if si is not None and len(si.on_wait) > max_waits:
                waits = list(si.on_wait)
                keep = waits[-max_waits:]
                rest = waits[:-max_waits]
                for i in range(0, len(rest), max_waits):
                    nop = mybir.InstNoOp(name=f"I-waitsplit-{uid}", ins=[],
                                         outs=[])
                    uid += 1
                    nop.engine = ins.engine
                    nop.sync_info = mybir.SyncInfo(
                        on_wait=rest[i:i + max_waits], on_update=[])
                    nc.register_instruction(nop)
                    out.append(nop)
                ins.sync_info = mybir.SyncInfo(
                    on_wait=keep, on_update=list(si.on_update))
            out.append(ins)
        bb.instructions = out


def _get_program():
    global _cached
    if _cached is None:
        _cached = _build_program()
    return _cached


def _preprocess(x, adj, W, a_src, a_dst):
    """Build the axis-0-concatenated global input arrays (what shard_map
    splits per core)."""
    xf = np.asarray(x, np.float32).astype(np.float16)
    xt = np.ascontiguousarray(
        xf.reshape(NCORES, R, F_IN).transpose(0, 2, 1)).reshape(N, R)
    adj = np.asarray(adj, np.int32)
    # adjacency is binary; the little-endian low byte of each int32 IS the
    # value, so packbits can run on a strided byte view (no bool temp)
    mb = np.packbits(adj.view(np.uint8)[:, ::4], axis=1)     # [N, QJ]
    wf = np.ascontiguousarray(np.asarray(W, np.float32)).astype(
        np.float16).reshape(N, D)
    asr = np.tile(np.asarray(a_src, np.float32).astype(np.float16)
                  .reshape(1, H * D), (NCORES, 1))
    adr = np.tile(np.asarray(a_dst, np.float32).astype(np.float16)
                  .reshape(1, H * D), (NCORES, 1))
    return {"xt": xt, "mb": mb, "wc": wf, "asrc": asr, "adst": adr}


def _get_exec():
    """Build (once) a persistent jitted shard_map executable around the Bass
    program — same lowering path as bass_utils.run_bass_kernel_spmd /
    bass2jax.run_bass_via_pjrt, but cached across calls so warm invocations
    skip re-trace/re-lowering."""
    global _exec_cached
    if _exec_cached is None:
        import jax
        from jax.experimental.shard_map import shard_map
        from jax.sharding import Mesh, PartitionSpec
        from concourse import mybir
        from concourse.bass2jax import (
            _bass_exec_p, install_neuronx_cc_hook, partition_id_tensor)

        install_neuronx_cc_hook()
        nc = _get_program()

        partition_name = (nc.partition_id_tensor.name
                          if nc.partition_id_tensor else None)
        in_names, out_names, out_avals, zero_shapes = [], [], [], []
        for alloc in nc.m.functions[0].allocations:
            if not isinstance(alloc, mybir.MemoryLocationSet):
                continue
            name = alloc.memorylocations[0].name
            if alloc.kind == "ExternalInput":
                if name != partition_name:
                    in_names.append(name)
            elif alloc.kind == "ExternalOutput":
                shape = tuple(alloc.tensor_shape)
                dtype = mybir.dt.np(alloc.dtype)
                out_names.append(name)
                out_avals.append(jax.core.ShapedArray(shape, dtype))
                zero_shapes.append(((NCORES * shape[0],) + shape[1:], dtype))
        n_params = len(in_names)
        all_names = list(in_names) + list(out_names)
        if partition_name is not None:
            all_names.append(partition_name)
        donate = tuple(range(n_params, n_params + len(out_names)))

        def _body(*args):
            operands = list(args)
            if partition_name is not None:
                operands.append(partition_id_tensor())
            return tuple(_bass_exec_p.bind(
                *operands,
                out_avals=tuple(out_avals),
                in_names=tuple(all_names),
                out_names=tuple(out_names),
                lowering_input_output_aliases=(),
                sim_require_finite=True,
                sim_require_nnan=True,
                nc=nc,
            ))

        devices = jax.devices()[:NCORES]
        mesh = Mesh(np.asarray(devices), ("core",))
        nin = n_params + len(out_names)
        sharded = jax.jit(
            shard_map(_body, mesh=mesh,
                      in_specs=(PartitionSpec("core"),) * nin,
                      out_specs=(PartitionSpec("core"),) * len(out_names),
                      check_rep=False),
            donate_argnums=donate, keep_unused=True)

        # donated output buffers made ON DEVICE (jnp.zeros) — shipping host
        # zeros would waste 4 MB of tunnel bandwidth per call
        import jax.numpy as jnp
        from jax.sharding import NamedSharding
        shardings = tuple(NamedSharding(mesh, PartitionSpec("core"))
                          for _ in zero_shapes)
        zeros_fn = jax.jit(
            lambda: tuple(jnp.zeros(s, d) for s, d in zero_shapes),
            out_shardings=shardings)
        _exec_cached = (sharded, in_names, out_names, zeros_fn)
    return _exec_cached


_zeros_next = None


def kernel(x, adj, W, a_src, a_dst):
    global _zeros_next
    import jax
    from jax.sharding import Mesh, NamedSharding, PartitionSpec

    sharded, in_names, out_names, zeros_fn = _get_exec()
    zeros = _zeros_next if _zeros_next is not None else zeros_fn()

    mesh = Mesh(np.asarray(jax.devices()[:NCORES]), ("core",))
    sh = NamedSharding(mesh, PartitionSpec("core"))

    # upload each input as soon as it is host-ready (device_put is async) so
    # tunnel transfer overlaps the remaining host-side packing
    dev = {}
    xf = np.asarray(x, np.float32).astype(np.float16)
    xt = np.ascontiguousarray(
        xf.reshape(NCORES, R, F_IN).transpose(0, 2, 1)).reshape(N, R)
    dev["xt"] = jax.device_put(xt, sh)
    wf = np.ascontiguousarray(np.asarray(W, np.float32)).astype(
        np.float16).reshape(N, D)
    dev["wc"] = jax.device_put(wf, sh)
    asr = np.tile(np.asarray(a_src, np.float32).astype(np.float16)
                  .reshape(1, H * D), (NCORES, 1))
    dev["asrc"] = jax.device_put(asr, sh)
    adr = np.tile(np.asarray(a_dst, np.float32).astype(np.float16)
                  .reshape(1, H * D), (NCORES, 1))
    dev["adst"] = jax.device_put(adr, sh)
    adj = np.asarray(adj, np.int32)
    mb = np.packbits(adj.view(np.uint8)[:, ::4], axis=1)
    dev["mb"] = jax.device_put(mb, sh)

    out_arrs = sharded(*[dev[n] for n in in_names], *zeros)
    _zeros_next = zeros_fn()  # async prefetch for the next call
    q = np.asarray(out_arrs[out_names.index("out")])
    sc = np.asarray(out_arrs[out_names.index("oscale")])
    out = (q.reshape(N, H, D).astype(np.float32)
           * sc.astype(np.float32)[:, :, None]).reshape(N, H * D)
    return out


# revision 25
# speedup vs baseline: 1.3207x; 1.3207x over previous
"""GAT (Cora-style) forward pass on 8 Trainium2 NeuronCores via Bass/Tile.

Sharding: row-parallel (sequence parallel over target nodes). Each core owns
N/8 = 512 target rows for ALL 8 heads. Tunnel traffic is minimized:
  - x is shipped transposed fp16, sharded by node rows (0.5 MB/core)
  - adj is shipped bit-packed along the source axis (64 KB... 256 KB/core)
  - W is shipped fp16 sharded by head (64 KB/core) and AllGather'd on device
  - per-core Wh / t are AllGather'd on device (no replicated host shipping)
Output comes back fp16 [512, 512] per core (0.5 MB/core).

Device pipeline per core (scores in [target r = partition, source j = free]):
  Wh_loc[r,(h,d)] = x_c @ W[h]            (PE, fp16 -> fp32 psum)
  AllGather Wh -> Wh_all[j,(h,d)] fp16 ; s/t = Wh . a_src/a_dst (DVE)
  AllGather t  -> t_all[j,h] fp32 ; T_b[h] = partition-broadcast of t (DMA)
  per (h, rt):      E1 = exp(T_b + s_r)   (ACT, bias=per-partition s column)
                    E2 = exp(0.2*T_b + 0.2*s_r)
                    P  = max(E1, E2)       == exp(leakyrelu(s+t, 0.2))
                    Pm = P * mask          (DVE, unpacked adj bits; bit value
                                            2^(7-k) is a per-j scale that
                                            cancels between num and denom)
                    per jt: pmT = PE-transpose(Pm chunk)
                            out[r,d] += pmT^T @ [Wh_all | 1]  (ones -> denom)
  epilogue: out = elu(num * recip(denom)) -> fp16
"""

import numpy as np
from contextlib import ExitStack

N = 4096
F_IN = 512
H = 8
D = 64
NCORES = 8
R = N // NCORES          # 512 target rows per core
P = 128                  # partitions
RT = R // P              # 4 row tiles per core
JT = N // P              # 32 source-node tiles
FT = F_IN // P           # 4 feature tiles
QJ = N // 8              # 512 packed bytes per target row

_cached = None
_exec_cached = None


def _build_program():
    import concourse.bass as bass
    import concourse.tile as tile
    from concourse import mybir
    from concourse.masks import make_identity

    f16, f32, u8 = mybir.dt.float16, mybir.dt.float32, mybir.dt.uint8
    i8 = mybir.dt.int8
    AF = mybir.ActivationFunctionType
    OP = mybir.AluOpType

    nc = bass.Bass(trn_type="TRN2", num_devices=NCORES)

    xt_in = nc.dram_tensor("xt", [F_IN, R], f16, kind="ExternalInput")
    mb_in = nc.dram_tensor("mb", [R, QJ], u8, kind="ExternalInput")
    wc_in = nc.dram_tensor("wc", [F_IN, D], f16, kind="ExternalInput")
    asrc_in = nc.dram_tensor("asrc", [1, H * D], f16, kind="ExternalInput")
    adst_in = nc.dram_tensor("adst", [1, H * D], f16, kind="ExternalInput")
    out_ext = nc.dram_tensor("out", [R, H * D], i8, kind="ExternalOutput")
    osc_ext = nc.dram_tensor("oscale", [R, H], f16, kind="ExternalOutput")

    w_loc = nc.dram_tensor("w_loc", [F_IN, D], f16)
    w_all = nc.dram_tensor("w_all", [N, D], f16, addr_space="Shared")
    wh_loc = nc.dram_tensor("wh_loc", [R, H * D], f16)
    wh_all = nc.dram_tensor("wh_all", [N, H * D], f16, addr_space="Shared")
    t_loc = nc.dram_tensor("t_loc", [H, R], f32)
    t_all = nc.dram_tensor("t_all", [NCORES * H, R], f32, addr_space="Shared")

    groups = [list(range(NCORES))]

    with tile.TileContext(nc) as tc:
        with ExitStack() as ctx:
            statics = ctx.enter_context(tc.tile_pool(name="statics", bufs=1))
            work = ctx.enter_context(tc.tile_pool(name="work", bufs=3))
            small = ctx.enter_context(tc.tile_pool(name="small", bufs=4))

            # ---- Phase A: W gather, Wh, s/t, mask unpack ----
            nc.gpsimd.dma_start(w_loc[:], wc_in[:])
            nc.gpsimd.collective_compute(
                "AllGather", OP.bypass, replica_groups=groups,
                ins=[w_loc[:]], outs=[w_all[:]],
            )

            xt_s = statics.tile([P, FT, R], f16)
            nc.sync.dma_start(xt_s[:], xt_in[:].rearrange("(ft p) r -> p ft r", p=P))
            w_s = statics.tile([P, H * FT, D], f16)
            nc.sync.dma_start(w_s[:], w_all[:].rearrange("(a p) d -> p a d", p=P))

            wh_s = statics.tile([P, RT, H * D], f16)

            with tc.tile_pool(name="psumA", bufs=2, space="PSUM") as psumA:
                for rt in range(RT):
                    for h in range(H):
                        pwh = psumA.tile([P, D], f32, tag="pwh")
                        for ft in range(FT):
                            nc.tensor.matmul(
                                pwh[:],
                                lhsT=xt_s[:, ft, rt * P:(rt + 1) * P],
                                rhs=w_s[:, h * FT + ft, :],
                                start=(ft == 0), stop=(ft == FT - 1),
                            )
                        nc.scalar.activation(
                            wh_s[:, rt, h * D:(h + 1) * D], pwh[:], AF.Copy)

                nc.sync.dma_start(
                    wh_loc[:].rearrange("(rt p) c -> p rt c", p=P), wh_s[:])
                nc.gpsimd.collective_compute(
                    "AllGather", OP.bypass, replica_groups=groups,
                    ins=[wh_loc[:]], outs=[wh_all[:]],
                )

                # broadcast a_src/a_dst across partitions via stride-0 DMA
                asb = statics.tile([P, H * D], f16)
                nc.gpsimd.dma_start(asb[:], asrc_in[:].broadcast_to([P, H * D]))
                adb = statics.tile([P, H * D], f16)
                nc.gpsimd.dma_start(adb[:], adst_in[:].broadcast_to([P, H * D]))

                # local s/t: multiply Wh by broadcast a, segment-reduce over d
                s_loc = statics.tile([P, RT, H], f32)
                t_loc_s = statics.tile([P, RT, H], f32)
                for rt in range(RT):
                    for ab, dst in ((asb, s_loc), (adb, t_loc_s)):
                        tmp = work.tile([P, H * D], f32, tag="sttmp", bufs=2)
                        nc.vector.tensor_mul(tmp[:], wh_s[:, rt, :], ab[:])
                        for h in range(H):
                            nc.vector.tensor_reduce(
                                dst[:, rt, h:h + 1], tmp[:, h * D:(h + 1) * D],
                                axis=mybir.AxisListType.X, op=OP.add)

                s2_loc = statics.tile([P, RT, H], f32)
                nc.vector.tensor_scalar(
                    out=s2_loc[:], in0=s_loc[:], scalar1=0.2, scalar2=None,
                    op0=OP.mult)

                for rt in range(RT):
                    nc.gpsimd.dma_start(
                        t_loc[:, rt * P:(rt + 1) * P].rearrange("h p -> p h"),
                        t_loc_s[:, rt, :])
                nc.gpsimd.collective_compute(
                    "AllGather", OP.bypass, replica_groups=groups,
                    ins=[t_loc[:]], outs=[t_all[:]],
                )

            # mask unpack: bit k of byte q -> source j = 8q + k, normalized
            # to exactly {0,1} via (byte & (1<<(7-k))) >> (7-k) — both ops
            # bitwise-class, which walrus accepts as a dual-op tensor_scalar.
            mb_s = statics.tile([P, RT, QJ], u8)
            nc.sync.dma_start(
                mb_s[:], mb_in[:].rearrange("(rt p) q -> p rt q", p=P))
            mask_s = statics.tile([P, RT, QJ, 8], f16)
            for rt in range(RT):
                mu = work.tile([P, QJ, 8], u8, tag="mu", bufs=2)
                for k in range(8):
                    nc.vector.tensor_scalar(
                        out=mu[:, :, k], in0=mb_s[:, rt, :],
                        scalar1=1 << (7 - k), scalar2=7 - k,
                        op0=OP.bitwise_and, op1=OP.logical_shift_right)
                nc.vector.tensor_copy(
                    mask_s[:, rt].rearrange("p a b -> p (a b)"),
                    mu[:].rearrange("p q k -> p (q k)"))

            # Wh_all + ones column -> whp
            whp = statics.tile([P, JT, H, D + 1], f16)
            wh_all_v = wh_all[:].rearrange("(jt p) (h d) -> p jt h d", p=P, d=D)
            for jt in range(JT):
                nc.sync.dma_start(whp[:, jt, :, 0:D], wh_all_v[:, jt])
            nc.vector.memset(whp[:, :, :, D:D + 1], 1.0)

            ident16 = statics.tile([P, P], f16)
            make_identity(nc, ident16)

            out_sb = [statics.tile([P, H * D], i8, name=f"outsb{rt}",
                                   tag=f"outsb{rt}") for rt in range(RT)]
            osc_sb = [statics.tile([P, H], f16, name=f"oscsb{rt}",
                                   tag=f"oscsb{rt}") for rt in range(RT)]

            # ---- Phase B: scores + aggregation ----
            with tc.tile_pool(name="psumB", bufs=2, space="PSUM") as psumB:
                for h in range(H):
                    tb = work.tile([P, N], f32, tag="tb", bufs=2)
                    t_all_v = t_all[:].rearrange("(c h) r -> h c r", h=H)
                    nc.gpsimd.dma_start(
                        tb[:].rearrange("p (c r) -> p c r", c=NCORES),
                        t_all_v[h:h + 1].broadcast_to([P, NCORES, R]))
                    for rt in range(RT):
                        e1 = work.tile([P, N], f16, tag="e1", bufs=2)
                        nc.scalar.activation(
                            e1[:], tb[:], AF.Exp,
                            bias=s_loc[:, rt, h:h + 1], scale=1.0)
                        e2 = work.tile([P, N], f16, tag="e2", bufs=2)
                        nc.scalar.activation(
                            e2[:], tb[:], AF.Exp,
                            bias=s2_loc[:, rt, h:h + 1], scale=0.2)
                        pm = work.tile([P, N], f16, tag="pm", bufs=2)
                        nc.vector.tensor_tensor(
                            out=pm[:], in0=e1[:], in1=e2[:], op=OP.max)
                        nc.vector.tensor_mul(
                            pm[:], pm[:],
                            mask_s[:, rt].rearrange("p a b -> p (a b)"))
                        po = psumB.tile([P, D + 1], f32, tag="po")
                        for jt in range(JT):
                            ptp = psumB.tile([P, P], f16, tag="ptp")
                            nc.tensor.transpose(
                                ptp[:], pm[:, jt * P:(jt + 1) * P], ident16[:])
                            pmT = work.tile([P, P], f16, tag="pmT")
                            nc.scalar.activation(pmT[:], ptp[:], AF.Copy)
                            nc.tensor.matmul(
                                po[:], lhsT=pmT[:], rhs=whp[:, jt, h, :],
                                start=(jt == 0), stop=(jt == JT - 1),
                            )
                        # epilogue: out = elu(num/denom)
                        ocp = small.tile([P, D + 1], f32, tag="ocp")
                        nc.scalar.activation(ocp[:], po[:], AF.Copy)
                        rec = small.tile([P, 1], f32, tag="rec")
                        nc.vector.reciprocal(rec[:], ocp[:, D:D + 1])
                        z = small.tile([P, D], f32, tag="z")
                        nc.vector.tensor_scalar(
                            out=z[:], in0=ocp[:, 0:D], scalar1=rec[:],
                            scalar2=None, op0=OP.mult)
                        zn = small.tile([P, D], f32, tag="zn")
                        nc.vector.tensor_scalar(
                            out=zn[:], in0=z[:], scalar1=0.0, scalar2=None,
                            op0=OP.min)
                        en = small.tile([P, D], f32, tag="en")
                        nc.scalar.activation(en[:], zn[:], AF.Exp)
                        zp = small.tile([P, D], f32, tag="zp")
                        nc.vector.tensor_scalar(
                            out=zp[:], in0=z[:], scalar1=0.0, scalar2=None,
                            op0=OP.max)
                        # of = (en - 1) + zp  == elu(z), fp32
                        of = small.tile([P, D], f32, tag="of")
                        nc.vector.scalar_tensor_tensor(
                            out=of[:], in0=en[:], scalar=-1.0, in1=zp[:],
                            op0=OP.add, op1=OP.add)
                        # per-(row,head) int8 quantization: q = of*126/amax,
                        # shipped with scale amax/126 (126 avoids saturation)
                        amax = small.tile([P, 1], f32, tag="amax")
                        nc.vector.tensor_reduce(
                            amax[:], of[:], axis=mybir.AxisListType.X,
                            op=OP.max, apply_absolute_value=True)
                        am2 = small.tile([P, 1], f32, tag="am2")
                        nc.vector.tensor_scalar(
                            out=am2[:], in0=amax[:], scalar1=1.0 / 126.0,
                            scalar2=1e-20, op0=OP.mult, op1=OP.max)
                        sinv = small.tile([P, 1], f32, tag="sinv")
                        nc.vector.reciprocal(sinv[:], am2[:])
                        nc.vector.tensor_scalar(
                            out=out_sb[rt][:, h * D:(h + 1) * D], in0=of[:],
                            scalar1=sinv[:], scalar2=None, op0=OP.mult)
                        nc.scalar.activation(
                            osc_sb[rt][:, h:h + 1], am2[:], AF.Copy)

            out_view = out_ext[:].rearrange("(rt p) c -> p rt c", p=P)
            osc_view = osc_ext[:].rearrange("(rt p) c -> p rt c", p=P)
            for rt in range(RT):
                nc.sync.dma_start(out_view[:, rt, :], out_sb[rt][:])
                nc.gpsimd.dma_start(osc_view[:, rt, :], osc_sb[rt][:])

    _legalize_waits(nc, mybir)
    return nc


def _legalize_waits(nc, mybir, max_waits=1):
    """walrus codegen allows very few sync-wait commands per instruction.
    Split surplus waits onto same-engine NoOps inserted just before the
    instruction — same wait point in that engine's program order."""
    uid = 0
    for bb in nc.main_func.blocks:
        out = []
        for ins in bb.instructions:
            si = ins.sync_info
            if si is not None and len(si.on_wait) > max_waits:
                waits = list(si.on_wait)
                keep = waits[-max_waits:]
                rest = waits[:-max_waits]
                for i in range(0, len(rest), max_waits):
                    nop = mybir.InstNoOp(name=f"I-waitsplit-{uid}", ins=[],
                                         outs=[])
                    uid += 1
                    nop.engine = ins.engine
                    nop.sync_info = mybir.SyncInfo(
                        on_wait=rest[i:i + max_waits], on_update=[])
                    nc.register_instruction(nop)
                    out.append(nop)
                ins.sync_info = mybir.SyncInfo(
                    on_wait=keep, on_update=list(si.on_update))
            out.append(ins)
        bb.instructions = out


def _get_program():
    global _cached
    if _cached is None:
        _cached = _build_program()
    return _cached


def _preprocess(x, adj, W, a_src, a_dst):
    """Build the axis-0-concatenated global input arrays (what shard_map
    splits per core)."""
    xf = np.asarray(x, np.float32).astype(np.float16)
    xt = np.ascontiguousarray(
        xf.reshape(NCORES, R, F_IN).transpose(0, 2, 1)).reshape(N, R)
    adj = np.asarray(adj, np.int32)
    # adjacency is binary; the little-endian low byte of each int32 IS the
    # value, so packbits can run on a strided byte view (no bool temp)
    mb = np.packbits(adj.view(np.uint8)[:, ::4], axis=1)     # [N, QJ]
    wf = np.ascontiguousarray(np.asarray(W, np.float32)).astype(
        np.float16).reshape(N, D)
    asr = np.tile(np.asarray(a_src, np.float32).astype(np.float16)
                  .reshape(1, H * D), (NCORES, 1))
    adr = np.tile(np.asarray(a_dst, np.float32).astype(np.float16)
                  .reshape(1, H * D), (NCORES, 1))
    return {"xt": xt, "mb": mb, "wc": wf, "asrc": asr, "adst": adr}


def _get_exec():
    """Build (once) a persistent jitted shard_map executable around the Bass
    program — same lowering path as bass_utils.run_bass_kernel_spmd /
    bass2jax.run_bass_via_pjrt, but cached across calls so warm invocations
    skip re-trace/re-lowering."""
    global _exec_cached
    if _exec_cached is None:
        import jax
        from jax.experimental.shard_map import shard_map
        from jax.sharding import Mesh, PartitionSpec
        from concourse import mybir
        from concourse.bass2jax import (
            _bass_exec_p, install_neuronx_cc_hook, partition_id_tensor)

        install_neuronx_cc_hook()
        nc = _get_program()

        partition_name = (nc.partition_id_tensor.name
                          if nc.partition_id_tensor else None)
        in_names, out_names, out_avals, zero_shapes = [], [], [], []
        for alloc in nc.m.functions[0].allocations:
            if not isinstance(alloc, mybir.MemoryLocationSet):
                continue
            name = alloc.memorylocations[0].name
            if alloc.kind == "ExternalInput":
                if name != partition_name:
                    in_names.append(name)
            elif alloc.kind == "ExternalOutput":
                shape = tuple(alloc.tensor_shape)
                dtype = mybir.dt.np(alloc.dtype)
                out_names.append(name)
                out_avals.append(jax.core.ShapedArray(shape, dtype))
                zero_shapes.append(((NCORES * shape[0],) + shape[1:], dtype))
        n_params = len(in_names)
        all_names = list(in_names) + list(out_names)
        if partition_name is not None:
            all_names.append(partition_name)
        donate = tuple(range(n_params, n_params + len(out_names)))

        def _body(*args):
            operands = list(args)
            if partition_name is not None:
                operands.append(partition_id_tensor())
            return tuple(_bass_exec_p.bind(
                *operands,
                out_avals=tuple(out_avals),
                in_names=tuple(all_names),
                out_names=tuple(out_names),
                lowering_input_output_aliases=(),
                sim_require_finite=True,
                sim_require_nnan=True,
                nc=nc,
            ))

        devices = jax.devices()[:NCORES]
        mesh = Mesh(np.asarray(devices), ("core",))
        nin = n_params + len(out_names)
        sharded = jax.jit(
            shard_map(_body, mesh=mesh,
                      in_specs=(PartitionSpec("core"),) * nin,
                      out_specs=(PartitionSpec("core"),) * len(out_names),
                      check_rep=False),
            donate_argnums=donate, keep_unused=True)

        # donated output buffers made ON DEVICE (jnp.zeros) — shipping host
        # zeros would waste 4 MB of tunnel bandwidth per call
        import jax.numpy as jnp
        from jax.sharding import NamedSharding
        shardings = tuple(NamedSharding(mesh, PartitionSpec("core"))
                          for _ in zero_shapes)
        zeros_fn = jax.jit(
            lambda: tuple(jnp.zeros(s, d) for s, d in zero_shapes),
            out_shardings=shardings)
        _exec_cached = (sharded, in_names, out_names, zeros_fn)
    return _exec_cached


_zeros_next = None


def kernel(x, adj, W, a_src, a_dst):
    global _zeros_next
    import jax
    from jax.sharding import Mesh, NamedSharding, PartitionSpec

    sharded, in_names, out_names, zeros_fn = _get_exec()
    zeros = _zeros_next if _zeros_next is not None else zeros_fn()

    mesh = Mesh(np.asarray(jax.devices()[:NCORES]), ("core",))
    sh = NamedSharding(mesh, PartitionSpec("core"))

    # upload each input as soon as it is host-ready (device_put is async) so
    # tunnel transfer overlaps the remaining host-side packing
    dev = {}
    xf = np.asarray(x, np.float32).astype(np.float16)
    xt = np.ascontiguousarray(
        xf.reshape(NCORES, R, F_IN).transpose(0, 2, 1)).reshape(N, R)
    dev["xt"] = jax.device_put(xt, sh)
    wf = np.ascontiguousarray(np.asarray(W, np.float32)).astype(
        np.float16).reshape(N, D)
    dev["wc"] = jax.device_put(wf, sh)
    asr = np.tile(np.asarray(a_src, np.float32).astype(np.float16)
                  .reshape(1, H * D), (NCORES, 1))
    dev["asrc"] = jax.device_put(asr, sh)
    adr = np.tile(np.asarray(a_dst, np.float32).astype(np.float16)
                  .reshape(1, H * D), (NCORES, 1))
    dev["adst"] = jax.device_put(adr, sh)
    adj = np.asarray(adj, np.int32)
    mb = np.packbits(adj.view(np.uint8)[:, ::4], axis=1)
    dev["mb"] = jax.device_put(mb, sh)

    out_arrs = sharded(*[dev[n] for n in in_names], *zeros)
    _zeros_next = zeros_fn()  # async prefetch for the next call
    q, sc = jax.device_get((out_arrs[out_names.index("out")],
                            out_arrs[out_names.index("oscale")]))
    out = (q.reshape(N, H, D).astype(np.float32)
           * sc.astype(np.float32)[:, :, None]).reshape(N, H * D)
    return out
